# revision 1
# baseline (speedup 1.0000x reference)
"""MoE (top-2 routing, 64 experts) on 8 Trainium2 NeuronCores.

Strategy (expert parallelism, per sharding hint):
  - Host computes the (tiny) router: logits = x @ w_router, softmax, top-2,
    renormalized gates. This is the dispatch decision = the sharding step.
  - Tokens are gathered per expert (capacity = max expert load, padded).
    Core m owns experts [8m, 8m+8) and receives its experts' gathered
    tokens in TRANSPOSED layout [H, C] plus the expert weights.
  - Device kernel (Bass/Tile, SPMD on cores 0-7): per expert,
    hT = gelu(w1.T @ xT + b1)  -> yT = w2.T @ hT  (all fp32, exact GELU).
    Transposed-activation layout means weights load in natural layout and
    no on-device transposes are needed anywhere.
  - Host combine: y = yT.T, out[tokens] += gate * (y + b2)  (b2 folded on
    host), plus the load-balance aux loss from the routing counts.

Compute/memory per core: 16.8 MB of expert weights (the dominant traffic,
~47us at ~360 GB/s) + ~27us of PE matmuls, overlapped.
"""

import numpy as np

B, S, H, E, F = 4, 1024, 256, 64, 1024
TOP_K = 2
T = B * S
N_CORES = 8
E_PER = E // N_CORES  # 8 experts per core
FT = F // 128  # 8 F-tiles
HT = H // 128  # 2 H-tiles

_compiled = {}


def _build(C, repeat=1):
    """Build + compile the per-core Bass program for capacity C."""
    import concourse.bass as bass  # noqa: F401
    import concourse.tile as tile
    import concourse.mybir as mybir
    from concourse import bacc
    from concourse.masks import make_identity

    f32 = mybir.dt.float32
    AF = mybir.ActivationFunctionType

    nc = bacc.Bacc("TRN2")
    xg = nc.dram_tensor("xg", [E_PER, HT, 128, C], f32, kind="ExternalInput")
    w1 = nc.dram_tensor("w1", [E_PER, HT, 128, F], f32, kind="ExternalInput")
    b1 = nc.dram_tensor("b1", [E_PER, FT, 128], f32, kind="ExternalInput")
    w2 = nc.dram_tensor("w2", [E_PER, FT, 128, H], f32, kind="ExternalInput")
    yT = nc.dram_tensor("yT", [E_PER, HT, 128, C], f32, kind="ExternalOutput")

    with tile.TileContext(nc) as tc:
        with (
            tc.tile_pool(name="const", bufs=1) as const,
            tc.tile_pool(name="w1p", bufs=2) as w1p,
            tc.tile_pool(name="w2p", bufs=2) as w2p,
            tc.tile_pool(name="xp", bufs=2) as xp,
            tc.tile_pool(name="hp", bufs=2) as hp,
            tc.tile_pool(name="op", bufs=4) as op,
            tc.tile_pool(name="psb", bufs=1, space="PSUM") as psb,
            tc.tile_pool(name="ps1", bufs=4, space="PSUM") as ps1,
            tc.tile_pool(name="ps2", bufs=2, space="PSUM") as ps2,
        ):
            # b1 arrives as [E_PER*FT, 128] rows; one PE transpose puts each
            # 128-wide bias row onto partitions: b1t[:, e*FT+f] is the [128,1]
            # per-partition bias column for F-tile f of expert e.
            ident = const.tile([E_PER * FT, E_PER * FT], f32)
            make_identity(nc, ident)
            b1rows = const.tile([E_PER * FT, 128], f32)
            nc.sync.dma_start(b1rows[:], b1.rearrange("e f p -> (e f) p"))
            b1ps = psb.tile([128, E_PER * FT], f32)
            nc.tensor.transpose(b1ps[:], b1rows[:], ident[:])
            b1t = const.tile([128, E_PER * FT], f32)
            nc.vector.tensor_copy(b1t[:], b1ps[:])

            for _ in range(repeat):
                for e in range(E_PER):
                    w1t = w1p.tile([128, HT, F], f32, tag="w1t")
                    nc.sync.dma_start(w1t[:], w1[e].rearrange("k p f -> p k f"))
                    xt = xp.tile([128, HT, C], f32, tag="xt")
                    nc.sync.dma_start(xt[:], xg[e].rearrange("k p c -> p k c"))
                    w2t = w2p.tile([128, FT, H], f32, tag="w2t")
                    nc.sync.dma_start(w2t[:], w2[e].rearrange("k p h -> p k h"))

                    # layer 1: hT[f*128:(f+1)*128, :] = gelu(w1.T @ xT + b1)
                    ht = hp.tile([128, FT, C], f32, tag="ht")
                    for f in range(FT):
                        ps = ps1.tile([128, C], f32, tag="ps1t")
                        for k in range(HT):
                            nc.tensor.matmul(
                                ps[:],
                                w1t[:, k, f * 128 : (f + 1) * 128],
                                xt[:, k],
                                start=(k == 0),
                                stop=(k == HT - 1),
                            )
                        nc.scalar.activation(
                            ht[:, f],
                            ps[:],
                            AF.Gelu,
                            bias=b1t[:, e * FT + f : e * FT + f + 1],
                        )

                    # layer 2: yT[h*128:(h+1)*128, :] = w2.T @ hT   (b2 on host)
                    for hh in range(HT):
                        pso = ps2.tile([128, C], f32, tag="ps2t")
                        for k in range(FT):
                            nc.tensor.matmul(
                                pso[:],
                                w2t[:, k, hh * 128 : (hh + 1) * 128],
                                ht[:, k],
                                start=(k == 0),
                                stop=(k == FT - 1),
                            )
                        ot = op.tile([128, C], f32, tag="ot")
                        nc.vector.tensor_copy(ot[:], pso[:])
                        nc.sync.dma_start(yT[e, hh], ot[:])

    nc.compile()
    return nc


def _get_program(C, repeat=1):
    key = (C, repeat)
    if key not in _compiled:
        _compiled[key] = _build(C, repeat)
    return _compiled[key]


def _route(xf, w_router, b_router):
    """Replicate reference routing in fp32 numpy. Returns top-2 idx/gates."""
    logits = xf @ w_router + b_router  # [T, E] fp32
    m = logits.max(axis=-1, keepdims=True)
    p = np.exp(logits - m, dtype=np.float32)
    p /= p.sum(axis=-1, keepdims=True)
    rows = np.arange(T)
    i1 = p.argmax(axis=-1)
    p1 = p[rows, i1]
    pm = p.copy()
    pm[rows, i1] = -np.inf
    i2 = pm.argmax(axis=-1)
    p2 = p[rows, i2]
    denom = p1 + p2
    g1 = p1 / denom
    g2 = p2 / denom
    return i1, i2, g1.astype(np.float32), g2.astype(np.float32)


def run_device(in_maps, C, repeat=1):
    from concourse.bass_utils import run_bass_kernel_spmd

    nc = _get_program(C, repeat)
    res = run_bass_kernel_spmd(nc, in_maps, list(range(N_CORES)))
    return res


def prepare(x, w_router, b_router, w1, b1, w2, b2):
    """Host routing + dispatch. Returns (in_maps, combine_state)."""
    x = np.ascontiguousarray(np.asarray(x, dtype=np.float32))
    w_router = np.asarray(w_router, dtype=np.float32)
    b_router = np.asarray(b_router, dtype=np.float32)
    w1 = np.ascontiguousarray(np.asarray(w1, dtype=np.float32))
    b1 = np.ascontiguousarray(np.asarray(b1, dtype=np.float32))
    w2 = np.ascontiguousarray(np.asarray(w2, dtype=np.float32))
    b2 = np.asarray(b2, dtype=np.float32)

    xf = x.reshape(T, H)
    i1, i2, g1, g2 = _route(xf, w_router, b_router)

    ee = np.concatenate([i1, i2])  # expert of each (token, slot)
    tok = np.concatenate([np.arange(T, dtype=np.int32)] * 2)
    gg = np.concatenate([g1, g2])
    order = np.argsort(ee, kind="stable")
    ee_s, tok_s, gg_s = ee[order], tok[order], gg[order]
    counts = np.bincount(ee, minlength=E)
    starts = np.zeros(E + 1, dtype=np.int64)
    np.cumsum(counts, out=starts[1:])

    cmax = int(counts.max())
    C = max(128, -(-cmax // 16) * 16)
    assert C <= 512, f"expert overflow: max count {cmax} > 512 unsupported"

    xg = np.zeros((E, H, C), dtype=np.float32)
    for e in range(E):
        lo, hi = starts[e], starts[e + 1]
        if hi > lo:
            xg[e, :, : hi - lo] = xf[tok_s[lo:hi]].T

    xg4 = xg.reshape(E, HT, 128, C)
    w1r = w1.reshape(E, HT, 128, F)
    b1r = b1.reshape(E, FT, 128)
    w2r = w2.reshape(E, FT, 128, H)

    in_maps = []
    for c in range(N_CORES):
        sl = slice(c * E_PER, (c + 1) * E_PER)
        in_maps.append(
            {
                "xg": np.ascontiguousarray(xg4[sl]),
                "w1": np.ascontiguousarray(w1r[sl]),
                "b1": np.ascontiguousarray(b1r[sl]),
                "w2": np.ascontiguousarray(w2r[sl]),
            }
        )
    state = dict(
        C=C, counts=counts, starts=starts, tok_s=tok_s, gg_s=gg_s, b2=b2, ee=ee
    )
    return in_maps, state


def combine(results, state):
    """Scatter-add device outputs back to [B,S,H] + aux loss."""
    C = state["C"]
    starts, tok_s, gg_s = state["starts"], state["tok_s"], state["gg_s"]
    b2 = state["b2"]
    out = np.zeros((T, H), dtype=np.float32)
    for c in range(N_CORES):
        yT = results[c]["yT"].reshape(E_PER, H, C)
        for j in range(E_PER):
            e = c * E_PER + j
            lo, hi = starts[e], starts[e + 1]
            if hi > lo:
                y = yT[j, :, : hi - lo].T  # [cnt, H]
                g = gg_s[lo:hi].astype(np.float32)[:, None]
                out[tok_s[lo:hi]] += g * (y + b2[e][None, :])
    output = out.reshape(B, S, H)

    usage = (state["counts"] / np.float32(2 * T)).astype(np.float32)
    target = np.full((E,), np.float32(1.0) / np.float32(E), dtype=np.float32)
    lb_loss = np.float32(np.mean((usage - target) ** 2, dtype=np.float32) * 0.01)
    return output, lb_loss


def kernel(x, w_router, b_router, w1, b1, w2, b2):
    in_maps, state = prepare(x, w_router, b_router, w1, b1, w2, b2)
    res = run_device(in_maps, state["C"], repeat=1)
    return combine(res.results, state)


# revision 4
# speedup vs baseline: 125.1917x; 125.1917x over previous
"""MoE (top-2 routing, 64 experts) on 8 Trainium2 NeuronCores.

Strategy (expert parallelism, per sharding hint):
  - Host computes the (tiny) router: logits = x @ w_router, softmax, top-2,
    renormalized gates. This is the dispatch decision = the sharding step.
  - Tokens are gathered per expert (capacity = max expert load, padded).
    Core m owns experts [8m, 8m+8) and receives its experts' gathered
    tokens in TRANSPOSED layout [H, C] plus the expert weights.
  - Device kernel (Bass/Tile, SPMD on cores 0-7): per expert,
    hT = gelu(w1.T @ xT + b1)  -> yT = w2.T @ hT  (all fp32, exact GELU).
    Transposed-activation layout means weights load in natural layout and
    no on-device transposes are needed anywhere.
  - Host combine: y = yT.T, out[tokens] += gate * (y + b2)  (b2 folded on
    host), plus the load-balance aux loss from the routing counts.

Compute/memory per core: 16.8 MB of expert weights (the dominant traffic,
~47us at ~360 GB/s) + ~27us of PE matmuls, overlapped.
"""

from contextlib import nullcontext

import numpy as np

B, S, H, E, F = 4, 1024, 256, 64, 1024
TOP_K = 2
T = B * S
N_CORES = 8
E_PER = E // N_CORES  # 8 experts per core
FT = F // 128  # 8 F-tiles
HT = H // 128  # 2 H-tiles

_compiled = {}


def _build(C, repeat=1, dyn_loop=False):
    """Build + compile the per-core Bass program for capacity C.

    repeat/dyn_loop repeat the whole expert pass (same data) for timing:
    dyn_loop uses a For_i hardware loop so code size stays constant.
    """
    import concourse.bass as bass  # noqa: F401
    import concourse.tile as tile
    import concourse.mybir as mybir
    from concourse import bacc
    from concourse.masks import make_identity

    f32 = mybir.dt.float32
    AF = mybir.ActivationFunctionType

    nc = bacc.Bacc("TRN2")
    xg = nc.dram_tensor("xg", [E_PER, HT, 128, C], f32, kind="ExternalInput")
    w1 = nc.dram_tensor("w1", [E_PER, HT, 128, F], f32, kind="ExternalInput")
    b1 = nc.dram_tensor("b1", [E_PER, FT, 128], f32, kind="ExternalInput")
    w2 = nc.dram_tensor("w2", [E_PER, FT, 128, H], f32, kind="ExternalInput")
    yT = nc.dram_tensor("yT", [E_PER, HT, 128, C], f32, kind="ExternalOutput")

    with tile.TileContext(nc) as tc:
        with (
            tc.tile_pool(name="const", bufs=1) as const,
            tc.tile_pool(name="w1p", bufs=2) as w1p,
            tc.tile_pool(name="w2p", bufs=2) as w2p,
            tc.tile_pool(name="xp", bufs=2) as xp,
            tc.tile_pool(name="hp", bufs=2) as hp,
            tc.tile_pool(name="op", bufs=4) as op,
            tc.tile_pool(name="psb", bufs=1, space="PSUM") as psb,
            tc.tile_pool(name="ps1", bufs=4, space="PSUM") as ps1,
            tc.tile_pool(name="ps2", bufs=2, space="PSUM") as ps2,
        ):
            # b1 arrives as [E_PER*FT, 128] rows; one PE transpose puts each
            # 128-wide bias row onto partitions: b1t[:, e*FT+f] is the [128,1]
            # per-partition bias column for F-tile f of expert e.
            ident = const.tile([E_PER * FT, E_PER * FT], f32)
            make_identity(nc, ident)
            b1rows = const.tile([E_PER * FT, 128], f32)
            nc.sync.dma_start(b1rows[:], b1.rearrange("e f p -> (e f) p"))
            b1ps = psb.tile([128, E_PER * FT], f32)
            nc.tensor.transpose(b1ps[:], b1rows[:], ident[:])
            b1t = const.tile([128, E_PER * FT], f32)
            nc.vector.tensor_copy(b1t[:], b1ps[:])

            def expert_pass():
                for e in range(E_PER):
                    w1t = w1p.tile([128, HT, F], f32, tag="w1t")
                    nc.sync.dma_start(w1t[:], w1[e].rearrange("k p f -> p k f"))
                    xt = xp.tile([128, HT, C], f32, tag="xt")
                    nc.sync.dma_start(xt[:], xg[e].rearrange("k p c -> p k c"))
                    w2t = w2p.tile([128, FT, H], f32, tag="w2t")
                    nc.sync.dma_start(w2t[:], w2[e].rearrange("k p h -> p k h"))

                    # layer 1: hT[f*128:(f+1)*128, :] = gelu(w1.T @ xT + b1)
                    ht = hp.tile([128, FT, C], f32, tag="ht")
                    for f in range(FT):
                        ps = ps1.tile([128, C], f32, tag="ps1t")
                        for k in range(HT):
                            nc.tensor.matmul(
                                ps[:],
                                w1t[:, k, f * 128 : (f + 1) * 128],
                                xt[:, k],
                                start=(k == 0),
                                stop=(k == HT - 1),
                            )
                        nc.scalar.activation(
                            ht[:, f],
                            ps[:],
                            AF.Gelu,
                            bias=b1t[:, e * FT + f : e * FT + f + 1],
                        )

                    # layer 2: yT[h*128:(h+1)*128, :] = w2.T @ hT  (b2 on host)
                    for hh in range(HT):
                        pso = ps2.tile([128, C], f32, tag="ps2t")
                        for k in range(FT):
                            nc.tensor.matmul(
                                pso[:],
                                w2t[:, k, hh * 128 : (hh + 1) * 128],
                                ht[:, k],
                                start=(k == 0),
                                stop=(k == FT - 1),
                            )
                        ot = op.tile([128, C], f32, tag="ot")
                        nc.vector.tensor_copy(ot[:], pso[:])
                        nc.sync.dma_start(yT[e, hh], ot[:])

            loop_ctx = tc.For_i(0, repeat, 1) if dyn_loop else nullcontext()
            with loop_ctx:
                for _ in range(1 if dyn_loop else repeat):
                    expert_pass()

    nc.compile()
    return nc


def _get_program(C, repeat=1, dyn_loop=False):
    key = (C, repeat, dyn_loop)
    if key not in _compiled:
        _compiled[key] = _build(C, repeat, dyn_loop)
    return _compiled[key]


def _route(xf, w_router, b_router):
    """Replicate reference routing in fp32 numpy. Returns top-2 idx/gates."""
    logits = xf @ w_router + b_router  # [T, E] fp32
    m = logits.max(axis=-1, keepdims=True)
    p = np.exp(logits - m, dtype=np.float32)
    p /= p.sum(axis=-1, keepdims=True)
    rows = np.arange(T)
    i1 = p.argmax(axis=-1)
    p1 = p[rows, i1]
    pm = p.copy()
    pm[rows, i1] = -np.inf
    i2 = pm.argmax(axis=-1)
    p2 = p[rows, i2]
    denom = p1 + p2
    g1 = p1 / denom
    g2 = p2 / denom
    return i1, i2, g1.astype(np.float32), g2.astype(np.float32)


def run_device(in_maps, C, repeat=1, dyn_loop=False):
    from concourse.bass_utils import run_bass_kernel_spmd

    nc = _get_program(C, repeat, dyn_loop)
    res = run_bass_kernel_spmd(nc, in_maps, list(range(N_CORES)))
    return res


def prepare(x, w_router, b_router, w1, b1, w2, b2):
    """Host routing + dispatch. Returns (in_maps, combine_state)."""
    x = np.ascontiguousarray(np.asarray(x, dtype=np.float32))
    w_router = np.asarray(w_router, dtype=np.float32)
    b_router = np.asarray(b_router, dtype=np.float32)
    w1 = np.ascontiguousarray(np.asarray(w1, dtype=np.float32))
    b1 = np.ascontiguousarray(np.asarray(b1, dtype=np.float32))
    w2 = np.ascontiguousarray(np.asarray(w2, dtype=np.float32))
    b2 = np.asarray(b2, dtype=np.float32)

    xf = x.reshape(T, H)
    i1, i2, g1, g2 = _route(xf, w_router, b_router)

    ee = np.concatenate([i1, i2])  # expert of each (token, slot)
    tok = np.concatenate([np.arange(T, dtype=np.int32)] * 2)
    gg = np.concatenate([g1, g2])
    order = np.argsort(ee, kind="stable")
    ee_s, tok_s, gg_s = ee[order], tok[order], gg[order]
    counts = np.bincount(ee, minlength=E)
    starts = np.zeros(E + 1, dtype=np.int64)
    np.cumsum(counts, out=starts[1:])

    cmax = int(counts.max())
    C = max(128, -(-cmax // 16) * 16)
    assert C <= 512, f"expert overflow: max count {cmax} > 512 unsupported"

    xg = np.zeros((E, H, C), dtype=np.float32)
    for e in range(E):
        lo, hi = starts[e], starts[e + 1]
        if hi > lo:
            xg[e, :, : hi - lo] = xf[tok_s[lo:hi]].T

    xg4 = xg.reshape(E, HT, 128, C)
    w1r = w1.reshape(E, HT, 128, F)
    b1r = b1.reshape(E, FT, 128)
    w2r = w2.reshape(E, FT, 128, H)

    in_maps = []
    for c in range(N_CORES):
        sl = slice(c * E_PER, (c + 1) * E_PER)
        in_maps.append(
            {
                "xg": np.ascontiguousarray(xg4[sl]),
                "w1": np.ascontiguousarray(w1r[sl]),
                "b1": np.ascontiguousarray(b1r[sl]),
                "w2": np.ascontiguousarray(w2r[sl]),
            }
        )
    state = dict(
        C=C, counts=counts, starts=starts, tok_s=tok_s, gg_s=gg_s, b2=b2, ee=ee
    )
    return in_maps, state


def combine(results, state):
    """Scatter-add device outputs back to [B,S,H] + aux loss."""
    C = state["C"]
    starts, tok_s, gg_s = state["starts"], state["tok_s"], state["gg_s"]
    b2 = state["b2"]
    out = np.zeros((T, H), dtype=np.float32)
    for c in range(N_CORES):
        yT = results[c]["yT"].reshape(E_PER, H, C)
        for j in range(E_PER):
            e = c * E_PER + j
            lo, hi = starts[e], starts[e + 1]
            if hi > lo:
                y = yT[j, :, : hi - lo].T  # [cnt, H]
                g = gg_s[lo:hi].astype(np.float32)[:, None]
                out[tok_s[lo:hi]] += g * (y + b2[e][None, :])
    output = out.reshape(B, S, H)

    usage = (state["counts"] / np.float32(2 * T)).astype(np.float32)
    target = np.full((E,), np.float32(1.0) / np.float32(E), dtype=np.float32)
    lb_loss = np.float32(np.mean((usage - target) ** 2, dtype=np.float32) * 0.01)
    return output, lb_loss


def kernel(x, w_router, b_router, w1, b1, w2, b2):
    in_maps, state = prepare(x, w_router, b_router, w1, b1, w2, b2)
    res = run_device(in_maps, state["C"], repeat=1)
    return combine(res.results, state)


# revision 5
# speedup vs baseline: 128.9959x; 1.0304x over previous
"""MoE (top-2 routing, 64 experts) on 8 Trainium2 NeuronCores.

Strategy (expert parallelism, per sharding hint):
  - Host computes the (tiny) router: logits = x @ w_router, softmax, top-2,
    renormalized gates. This is the dispatch decision = the sharding step.
  - Tokens are gathered per expert (capacity = max expert load, padded).
    Core m owns experts [8m, 8m+8) and receives its experts' gathered
    tokens in TRANSPOSED layout [H, C] plus the expert weights.
  - Device kernel (Bass/Tile, SPMD on cores 0-7): per expert,
    hT = gelu(w1.T @ xT + b1)  -> yT = w2.T @ hT  (all fp32, exact GELU).
    Transposed-activation layout means weights need no on-device transposes.
  - Host combine: y = yT.T, out[tokens] += gate * (y + b2)  (b2 folded on
    host), plus the load-balance aux loss from the routing counts.

All DRAM tensors are host-pre-packed so every DMA reads/writes >=4KB
contiguous per partition (1KB-chunk patterns measured at ~136 GB/s vs
~340 GB/s for >=4KB):
  w1  [E_PER, 128, HT, F]   per-partition run 8KB
  w2  [E_PER, 128, FT, H]   per-partition run 8KB
  xg  [128, E_PER, HT, C]   one load for all experts, run ~13KB
  yT  [128, E_PER, HT, C]   one store at end of pass
"""

from contextlib import nullcontext

import numpy as np

B, S, H, E, F = 4, 1024, 256, 64, 1024
TOP_K = 2
T = B * S
N_CORES = 8
E_PER = E // N_CORES  # 8 experts per core
FT = F // 128  # 8 F-tiles
HT = H // 128  # 2 H-tiles

_compiled = {}


def _build(C, repeat=1, dyn_loop=False):
    """Build + compile the per-core Bass program for capacity C.

    repeat/dyn_loop repeat the whole expert pass (same data) for timing:
    dyn_loop uses a For_i hardware loop so code size stays constant.
    """
    import concourse.bass as bass  # noqa: F401
    import concourse.tile as tile
    import concourse.mybir as mybir
    from concourse import bacc
    from concourse.masks import make_identity

    f32 = mybir.dt.float32
    AF = mybir.ActivationFunctionType

    nc = bacc.Bacc("TRN2")
    xg = nc.dram_tensor("xg", [128, E_PER, HT, C], f32, kind="ExternalInput")
    w1 = nc.dram_tensor("w1", [E_PER, 128, HT, F], f32, kind="ExternalInput")
    b1 = nc.dram_tensor("b1", [E_PER, FT, 128], f32, kind="ExternalInput")
    w2 = nc.dram_tensor("w2", [E_PER, 128, FT, H], f32, kind="ExternalInput")
    yT = nc.dram_tensor("yT", [128, E_PER, HT, C], f32, kind="ExternalOutput")

    with tile.TileContext(nc) as tc:
        with (
            tc.tile_pool(name="const", bufs=1) as const,
            tc.tile_pool(name="w1p", bufs=2) as w1p,
            tc.tile_pool(name="w2p", bufs=2) as w2p,
            tc.tile_pool(name="xp", bufs=2) as xp,
            tc.tile_pool(name="hp", bufs=2) as hp,
            tc.tile_pool(name="op", bufs=2) as op,
            tc.tile_pool(name="psb", bufs=1, space="PSUM") as psb,
            tc.tile_pool(name="ps1", bufs=4, space="PSUM") as ps1,
            tc.tile_pool(name="ps2", bufs=2, space="PSUM") as ps2,
        ):
            # b1 arrives as [E_PER*FT, 128] rows; one PE transpose puts each
            # 128-wide bias row onto partitions: b1t[:, e*FT+f] is the [128,1]
            # per-partition bias column for F-tile f of expert e.
            ident = const.tile([E_PER * FT, E_PER * FT], f32)
            make_identity(nc, ident)
            b1rows = const.tile([E_PER * FT, 128], f32)
            nc.sync.dma_start(b1rows[:], b1.rearrange("e f p -> (e f) p"))
            b1ps = psb.tile([128, E_PER * FT], f32)
            nc.tensor.transpose(b1ps[:], b1rows[:], ident[:])
            b1t = const.tile([128, E_PER * FT], f32)
            nc.vector.tensor_copy(b1t[:], b1ps[:])

            def expert_pass():
                # one consolidated token load / output store per pass
                xall = xp.tile([128, E_PER, HT, C], f32, tag="xall")
                nc.sync.dma_start(xall[:], xg[:])
                oall = op.tile([128, E_PER, HT, C], f32, tag="oall")
                for e in range(E_PER):
                    w1t = w1p.tile([128, HT, F], f32, tag="w1t")
                    nc.sync.dma_start(w1t[:], w1[e])
                    w2t = w2p.tile([128, FT, H], f32, tag="w2t")
                    nc.sync.dma_start(w2t[:], w2[e])

                    # layer 1: hT[f*128:(f+1)*128, :] = gelu(w1.T @ xT + b1)
                    ht = hp.tile([128, FT, C], f32, tag="ht")
                    for f in range(FT):
                        ps = ps1.tile([128, C], f32, tag="ps1t")
                        for k in range(HT):
                            nc.tensor.matmul(
                                ps[:],
                                w1t[:, k, f * 128 : (f + 1) * 128],
                                xall[:, e, k],
                                start=(k == 0),
                                stop=(k == HT - 1),
                            )
                        nc.scalar.activation(
                            ht[:, f],
                            ps[:],
                            AF.Gelu,
                            bias=b1t[:, e * FT + f : e * FT + f + 1],
                        )

                    # layer 2: yT[h*128:(h+1)*128, :] = w2.T @ hT  (b2 on host)
                    for hh in range(HT):
                        pso = ps2.tile([128, C], f32, tag="ps2t")
                        for k in range(FT):
                            nc.tensor.matmul(
                                pso[:],
                                w2t[:, k, hh * 128 : (hh + 1) * 128],
                                ht[:, k],
                                start=(k == 0),
                                stop=(k == FT - 1),
                            )
                        nc.vector.tensor_copy(oall[:, e, hh], pso[:])
                nc.sync.dma_start(yT[:], oall[:])

            loop_ctx = tc.For_i(0, repeat, 1) if dyn_loop else nullcontext()
            with loop_ctx:
                for _ in range(1 if dyn_loop else repeat):
                    expert_pass()

    nc.compile()
    return nc


def _get_program(C, repeat=1, dyn_loop=False):
    key = (C, repeat, dyn_loop)
    if key not in _compiled:
        _compiled[key] = _build(C, repeat, dyn_loop)
    return _compiled[key]


def _route(xf, w_router, b_router):
    """Replicate reference routing in fp32 numpy. Returns top-2 idx/gates."""
    logits = xf @ w_router + b_router  # [T, E] fp32
    m = logits.max(axis=-1, keepdims=True)
    p = np.exp(logits - m, dtype=np.float32)
    p /= p.sum(axis=-1, keepdims=True)
    rows = np.arange(T)
    i1 = p.argmax(axis=-1)
    p1 = p[rows, i1]
    pm = p.copy()
    pm[rows, i1] = -np.inf
    i2 = pm.argmax(axis=-1)
    p2 = p[rows, i2]
    denom = p1 + p2
    g1 = p1 / denom
    g2 = p2 / denom
    return i1, i2, g1.astype(np.float32), g2.astype(np.float32)


def run_device(in_maps, C, repeat=1, dyn_loop=False):
    from concourse.bass_utils import run_bass_kernel_spmd

    nc = _get_program(C, repeat, dyn_loop)
    res = run_bass_kernel_spmd(nc, in_maps, list(range(N_CORES)))
    return res


def prepare(x, w_router, b_router, w1, b1, w2, b2):
    """Host routing + dispatch. Returns (in_maps, combine_state)."""
    x = np.ascontiguousarray(np.asarray(x, dtype=np.float32))
    w_router = np.asarray(w_router, dtype=np.float32)
    b_router = np.asarray(b_router, dtype=np.float32)
    w1 = np.ascontiguousarray(np.asarray(w1, dtype=np.float32))
    b1 = np.ascontiguousarray(np.asarray(b1, dtype=np.float32))
    w2 = np.ascontiguousarray(np.asarray(w2, dtype=np.float32))
    b2 = np.asarray(b2, dtype=np.float32)

    xf = x.reshape(T, H)
    i1, i2, g1, g2 = _route(xf, w_router, b_router)

    ee = np.concatenate([i1, i2])  # expert of each (token, slot)
    tok = np.concatenate([np.arange(T, dtype=np.int32)] * 2)
    gg = np.concatenate([g1, g2])
    order = np.argsort(ee, kind="stable")
    ee_s, tok_s, gg_s = ee[order], tok[order], gg[order]
    counts = np.bincount(ee, minlength=E)
    starts = np.zeros(E + 1, dtype=np.int64)
    np.cumsum(counts, out=starts[1:])

    cmax = int(counts.max())
    C = max(128, -(-cmax // 16) * 16)
    assert C <= 512, f"expert overflow: max count {cmax} > 512 unsupported"

    xg = np.zeros((E, H, C), dtype=np.float32)
    for e in range(E):
        lo, hi = starts[e], starts[e + 1]
        if hi > lo:
            xg[e, :, : hi - lo] = xf[tok_s[lo:hi]].T

    # pack for >=4KB-contiguous-per-partition DMAs (see module docstring)
    w1p = w1.reshape(E, HT, 128, F).transpose(0, 2, 1, 3)  # [E,128,HT,F]
    w2p = w2.reshape(E, FT, 128, H).transpose(0, 2, 1, 3)  # [E,128,FT,H]
    b1r = b1.reshape(E, FT, 128)
    # xg per core: [128, E_PER, HT, C]
    xg5 = xg.reshape(E, HT, 128, C)

    in_maps = []
    for c in range(N_CORES):
        sl = slice(c * E_PER, (c + 1) * E_PER)
        in_maps.append(
            {
                "xg": np.ascontiguousarray(xg5[sl].transpose(2, 0, 1, 3)),
                "w1": np.ascontiguousarray(w1p[sl]),
                "b1": np.ascontiguousarray(b1r[sl]),
                "w2": np.ascontiguousarray(w2p[sl]),
            }
        )
    state = dict(
        C=C, counts=counts, starts=starts, tok_s=tok_s, gg_s=gg_s, b2=b2, ee=ee
    )
    return in_maps, state


def combine(results, state):
    """Scatter-add device outputs back to [B,S,H] + aux loss."""
    C = state["C"]
    starts, tok_s, gg_s = state["starts"], state["tok_s"], state["gg_s"]
    b2 = state["b2"]
    out = np.zeros((T, H), dtype=np.float32)
    for c in range(N_CORES):
        # yT [128, E_PER, HT, C] -> [E_PER, H, C]
        yTc = results[c]["yT"].transpose(1, 2, 0, 3).reshape(E_PER, H, C)
        for j in range(E_PER):
            e = c * E_PER + j
            lo, hi = starts[e], starts[e + 1]
            if hi > lo:
                y = yTc[j, :, : hi - lo].T  # [cnt, H]
                g = gg_s[lo:hi].astype(np.float32)[:, None]
                out[tok_s[lo:hi]] += g * (y + b2[e][None, :])
    output = out.reshape(B, S, H)

    usage = (state["counts"] / np.float32(2 * T)).astype(np.float32)
    target = np.full((E,), np.float32(1.0) / np.float32(E), dtype=np.float32)
    lb_loss = np.float32(np.mean((usage - target) ** 2, dtype=np.float32) * 0.01)
    return output, lb_loss


def kernel(x, w_router, b_router, w1, b1, w2, b2):
    in_maps, state = prepare(x, w_router, b_router, w1, b1, w2, b2)
    res = run_device(in_maps, state["C"], repeat=1)
    return combine(res.results, state)


# revision 6
# speedup vs baseline: 329.5059x; 2.5544x over previous
"""MoE (top-2 routing, 64 experts) on 8 Trainium2 NeuronCores.

Strategy (expert parallelism, per sharding hint):
  - Host computes the (tiny) router: logits = x @ w_router, softmax, top-2,
    renormalized gates. This is the dispatch decision = the sharding step.
  - Tokens are gathered per expert (capacity = max expert load, padded).
    Core m owns experts [8m, 8m+8) and receives its experts' gathered
    tokens in TRANSPOSED layout [H, C] plus the expert weights.
  - Device kernel (Bass/Tile, SPMD on cores 0-7): per expert,
    hT = gelu(w1.T @ xT + b1)  -> yT = w2.T @ hT  (all fp32, exact GELU).
    Transposed-activation layout means weights need no on-device transposes.
  - Host combine: y = yT.T, out[tokens] += gate * (y + b2)  (b2 folded on
    host), plus the load-balance aux loss from the routing counts.

All DRAM tensors are host-pre-packed so every DMA reads/writes >=4KB
contiguous per partition (1KB-chunk patterns measured at ~136 GB/s vs
~340 GB/s for >=4KB):
  w1  [E_PER, 128, HT, F]   per-partition run 8KB
  w2  [E_PER, 128, FT, H]   per-partition run 8KB
  xg  [128, E_PER, HT, C]   one load for all experts, run ~13KB
  yT  [128, E_PER, HT, C]   one store at end of pass

Matmul operands (weights, tokens, hidden) are fp16: the PE streams 2-byte
dtypes 3.25x faster than fp32 (139 vs 452 ns per [128,128]x[128,192] MM,
measured) and fp16's 10 mantissa bits keep the end-to-end error at ~4e-4
relative (PSUM accumulation and the output stay fp32; gelu is applied to
the fp32 pre-activation). Also halves the weight DMA traffic.
"""

from contextlib import nullcontext

import numpy as np

B, S, H, E, F = 4, 1024, 256, 64, 1024
TOP_K = 2
T = B * S
N_CORES = 8
E_PER = E // N_CORES  # 8 experts per core
FT = F // 128  # 8 F-tiles
HT = H // 128  # 2 H-tiles

_compiled = {}


def _build(C, repeat=1, dyn_loop=False):
    """Build + compile the per-core Bass program for capacity C.

    repeat/dyn_loop repeat the whole expert pass (same data) for timing:
    dyn_loop uses a For_i hardware loop so code size stays constant.
    """
    import concourse.bass as bass  # noqa: F401
    import concourse.tile as tile
    import concourse.mybir as mybir
    from concourse import bacc
    from concourse.masks import make_identity

    f32 = mybir.dt.float32
    f16 = mybir.dt.float16
    AF = mybir.ActivationFunctionType

    nc = bacc.Bacc("TRN2")
    xg = nc.dram_tensor("xg", [128, E_PER, HT, C], f16, kind="ExternalInput")
    w1 = nc.dram_tensor("w1", [E_PER, 128, HT, F], f16, kind="ExternalInput")
    b1 = nc.dram_tensor("b1", [E_PER, FT, 128], f32, kind="ExternalInput")
    w2 = nc.dram_tensor("w2", [E_PER, 128, FT, H], f16, kind="ExternalInput")
    yT = nc.dram_tensor("yT", [128, E_PER, HT, C], f32, kind="ExternalOutput")

    with tile.TileContext(nc) as tc:
        with (
            tc.tile_pool(name="const", bufs=1) as const,
            tc.tile_pool(name="w1p", bufs=2) as w1p,
            tc.tile_pool(name="w2p", bufs=2) as w2p,
            tc.tile_pool(name="xp", bufs=2) as xp,
            tc.tile_pool(name="hp", bufs=2) as hp,
            tc.tile_pool(name="op", bufs=2) as op,
            tc.tile_pool(name="psb", bufs=1, space="PSUM") as psb,
            tc.tile_pool(name="ps1", bufs=4, space="PSUM") as ps1,
            tc.tile_pool(name="ps2", bufs=2, space="PSUM") as ps2,
        ):
            # b1 arrives as [E_PER*FT, 128] rows; one PE transpose puts each
            # 128-wide bias row onto partitions: b1t[:, e*FT+f] is the [128,1]
            # per-partition bias column for F-tile f of expert e.
            ident = const.tile([E_PER * FT, E_PER * FT], f32)
            make_identity(nc, ident)
            b1rows = const.tile([E_PER * FT, 128], f32)
            nc.sync.dma_start(b1rows[:], b1.rearrange("e f p -> (e f) p"))
            b1ps = psb.tile([128, E_PER * FT], f32)
            nc.tensor.transpose(b1ps[:], b1rows[:], ident[:])
            b1t = const.tile([128, E_PER * FT], f32)
            nc.vector.tensor_copy(b1t[:], b1ps[:])

            def expert_pass():
                # one consolidated token load / output store per pass
                xall = xp.tile([128, E_PER, HT, C], f16, tag="xall")
                nc.sync.dma_start(xall[:], xg[:])
                oall = op.tile([128, E_PER, HT, C], f32, tag="oall")
                for e in range(E_PER):
                    w1t = w1p.tile([128, HT, F], f16, tag="w1t")
                    nc.sync.dma_start(w1t[:], w1[e])
                    w2t = w2p.tile([128, FT, H], f16, tag="w2t")
                    nc.sync.dma_start(w2t[:], w2[e])

                    # layer 1: hT[f*128:(f+1)*128, :] = gelu(w1.T @ xT + b1)
                    ht = hp.tile([128, FT, C], f16, tag="ht")
                    for f in range(FT):
                        ps = ps1.tile([128, C], f32, tag="ps1t")
                        for k in range(HT):
                            nc.tensor.matmul(
                                ps[:],
                                w1t[:, k, f * 128 : (f + 1) * 128],
                                xall[:, e, k],
                                start=(k == 0),
                                stop=(k == HT - 1),
                            )
                        nc.scalar.activation(
                            ht[:, f],
                            ps[:],
                            AF.Gelu,
                            bias=b1t[:, e * FT + f : e * FT + f + 1],
                        )

                    # layer 2: yT[h*128:(h+1)*128, :] = w2.T @ hT  (b2 on host)
                    for hh in range(HT):
                        pso = ps2.tile([128, C], f32, tag="ps2t")
                        for k in range(FT):
                            nc.tensor.matmul(
                                pso[:],
                                w2t[:, k, hh * 128 : (hh + 1) * 128],
                                ht[:, k],
                                start=(k == 0),
                                stop=(k == FT - 1),
                            )
                        nc.vector.tensor_copy(oall[:, e, hh], pso[:])
                nc.sync.dma_start(yT[:], oall[:])

            loop_ctx = tc.For_i(0, repeat, 1) if dyn_loop else nullcontext()
            with loop_ctx:
                for _ in range(1 if dyn_loop else repeat):
                    expert_pass()

    nc.compile()
    return nc


def _get_program(C, repeat=1, dyn_loop=False):
    key = (C, repeat, dyn_loop)
    if key not in _compiled:
        _compiled[key] = _build(C, repeat, dyn_loop)
    return _compiled[key]


def _route(xf, w_router, b_router):
    """Replicate reference routing in fp32 numpy. Returns top-2 idx/gates."""
    logits = xf @ w_router + b_router  # [T, E] fp32
    m = logits.max(axis=-1, keepdims=True)
    p = np.exp(logits - m, dtype=np.float32)
    p /= p.sum(axis=-1, keepdims=True)
    rows = np.arange(T)
    i1 = p.argmax(axis=-1)
    p1 = p[rows, i1]
    pm = p.copy()
    pm[rows, i1] = -np.inf
    i2 = pm.argmax(axis=-1)
    p2 = p[rows, i2]
    denom = p1 + p2
    g1 = p1 / denom
    g2 = p2 / denom
    return i1, i2, g1.astype(np.float32), g2.astype(np.float32)


def run_device(in_maps, C, repeat=1, dyn_loop=False):
    from concourse.bass_utils import run_bass_kernel_spmd

    nc = _get_program(C, repeat, dyn_loop)
    res = run_bass_kernel_spmd(nc, in_maps, list(range(N_CORES)))
    return res


def prepare(x, w_router, b_router, w1, b1, w2, b2):
    """Host routing + dispatch. Returns (in_maps, combine_state)."""
    x = np.ascontiguousarray(np.asarray(x, dtype=np.float32))
    w_router = np.asarray(w_router, dtype=np.float32)
    b_router = np.asarray(b_router, dtype=np.float32)
    w1 = np.ascontiguousarray(np.asarray(w1, dtype=np.float32))
    b1 = np.ascontiguousarray(np.asarray(b1, dtype=np.float32))
    w2 = np.ascontiguousarray(np.asarray(w2, dtype=np.float32))
    b2 = np.asarray(b2, dtype=np.float32)

    xf = x.reshape(T, H)
    i1, i2, g1, g2 = _route(xf, w_router, b_router)

    ee = np.concatenate([i1, i2])  # expert of each (token, slot)
    tok = np.concatenate([np.arange(T, dtype=np.int32)] * 2)
    gg = np.concatenate([g1, g2])
    order = np.argsort(ee, kind="stable")
    ee_s, tok_s, gg_s = ee[order], tok[order], gg[order]
    counts = np.bincount(ee, minlength=E)
    starts = np.zeros(E + 1, dtype=np.int64)
    np.cumsum(counts, out=starts[1:])

    cmax = int(counts.max())
    C = max(128, -(-cmax // 16) * 16)
    assert C <= 512, f"expert overflow: max count {cmax} > 512 unsupported"

    xg = np.zeros((E, H, C), dtype=np.float32)
    for e in range(E):
        lo, hi = starts[e], starts[e + 1]
        if hi > lo:
            xg[e, :, : hi - lo] = xf[tok_s[lo:hi]].T

    # pack for >=4KB-contiguous-per-partition DMAs (see module docstring)
    w1p = w1.reshape(E, HT, 128, F).transpose(0, 2, 1, 3)  # [E,128,HT,F]
    w2p = w2.reshape(E, FT, 128, H).transpose(0, 2, 1, 3)  # [E,128,FT,H]
    b1r = b1.reshape(E, FT, 128)
    # xg per core: [128, E_PER, HT, C]
    xg5 = xg.reshape(E, HT, 128, C)

    in_maps = []
    for c in range(N_CORES):
        sl = slice(c * E_PER, (c + 1) * E_PER)
        in_maps.append(
            {
                "xg": np.ascontiguousarray(
                    xg5[sl].transpose(2, 0, 1, 3).astype(np.float16)
                ),
                "w1": np.ascontiguousarray(w1p[sl].astype(np.float16)),
                "b1": np.ascontiguousarray(b1r[sl]),
                "w2": np.ascontiguousarray(w2p[sl].astype(np.float16)),
            }
        )
    state = dict(
        C=C, counts=counts, starts=starts, tok_s=tok_s, gg_s=gg_s, b2=b2, ee=ee
    )
    return in_maps, state


def combine(results, state):
    """Scatter-add device outputs back to [B,S,H] + aux loss."""
    C = state["C"]
    starts, tok_s, gg_s = state["starts"], state["tok_s"], state["gg_s"]
    b2 = state["b2"]
    out = np.zeros((T, H), dtype=np.float32)
    for c in range(N_CORES):
        # yT [128, E_PER, HT, C] -> [E_PER, H, C]
        yTc = results[c]["yT"].transpose(1, 2, 0, 3).reshape(E_PER, H, C)
        for j in range(E_PER):
            e = c * E_PER + j
            lo, hi = starts[e], starts[e + 1]
            if hi > lo:
                y = yTc[j, :, : hi - lo].T  # [cnt, H]
                g = gg_s[lo:hi].astype(np.float32)[:, None]
                out[tok_s[lo:hi]] += g * (y + b2[e][None, :])
    output = out.reshape(B, S, H)

    usage = (state["counts"] / np.float32(2 * T)).astype(np.float32)
    target = np.full((E,), np.float32(1.0) / np.float32(E), dtype=np.float32)
    lb_loss = np.float32(np.mean((usage - target) ** 2, dtype=np.float32) * 0.01)
    return output, lb_loss


def kernel(x, w_router, b_router, w1, b1, w2, b2):
    in_maps, state = prepare(x, w_router, b_router, w1, b1, w2, b2)
    res = run_device(in_maps, state["C"], repeat=1)
    return combine(res.results, state)


# revision 7
# speedup vs baseline: 426.6854x; 1.2949x over previous
"""MoE (top-2 routing, 64 experts) on 8 Trainium2 NeuronCores.

Strategy (expert parallelism, per sharding hint):
  - Host computes the (tiny) router: logits = x @ w_router, softmax, top-2,
    renormalized gates. This is the dispatch decision = the sharding step.
  - Experts are snake-dealt to (core, slot) by descending token count so
    slot s holds similarly-loaded experts on every core; the per-slot
    capacity Cs = max count in that slot (SPMD: one program, all cores).
  - Tokens are gathered per expert in TRANSPOSED layout [H, Cs]; core m
    receives its 8 experts' gathered tokens plus the expert weights.
  - Device kernel (Bass/Tile, SPMD on cores 0-7): per expert,
    hT = gelu(w1.T @ xT + b1)  ->  yT = w2.T @ hT.
    Transposed-activation layout: weights need no on-device transposes.
  - Host combine: y = yT.T, out[tokens] += gate * (y + b2)  (b2 folded on
    host), plus the load-balance aux loss from the routing counts.

Matmul operands (weights, tokens, hidden) are fp16: the PE streams 2-byte
dtypes 3.25x faster than fp32 (139 vs 452 ns per [128,128]x[128,192] MM,
measured) and fp16's 10 mantissa bits keep the end-to-end error at ~4e-4
relative (PSUM accumulation and the output stay fp32; gelu is applied to
the fp32 pre-activation). fp16 also halves the weight DMA traffic.

All DRAM tensors are host-pre-packed so every DMA reads/writes >=4KB
contiguous per partition (1KB-chunk patterns measured at ~136 GB/s vs
~340 GB/s for >=4KB):
  w1  [E_PER, 128, HT, F]  fp16, per-partition run 4KB
  w2  [E_PER, 128, FT, H]  fp16, per-partition run 4KB
  xg  [128, HT, L]         fp16, one load for all experts (L = sum Cs)
  yT  [128, HT, L]         fp32, one store at end of pass
"""

from contextlib import nullcontext

import numpy as np

B, S, H, E, F = 4, 1024, 256, 64, 1024
TOP_K = 2
T = B * S
N_CORES = 8
E_PER = E // N_CORES  # 8 experts per core
FT = F // 128  # 8 F-tiles
HT = H // 128  # 2 H-tiles

_compiled = {}


def _build(cs, repeat=1, dyn_loop=False):
    """Build + compile the per-core Bass program for slot capacities cs.

    repeat/dyn_loop repeat the whole expert pass (same data) for timing:
    dyn_loop uses a For_i hardware loop so code size stays constant.
    """
    import concourse.bass as bass  # noqa: F401
    import concourse.tile as tile
    import concourse.mybir as mybir
    from concourse import bacc
    from concourse.masks import make_identity

    f32 = mybir.dt.float32
    f16 = mybir.dt.float16
    AF = mybir.ActivationFunctionType

    cmax = max(cs)
    off = [0]
    for c in cs:
        off.append(off[-1] + c)
    L = off[-1]

    nc = bacc.Bacc("TRN2")
    xg = nc.dram_tensor("xg", [128, HT, L], f16, kind="ExternalInput")
    w1 = nc.dram_tensor("w1", [E_PER, 128, HT, F], f16, kind="ExternalInput")
    b1 = nc.dram_tensor("b1", [E_PER, FT, 128], f32, kind="ExternalInput")
    w2 = nc.dram_tensor("w2", [E_PER, 128, FT, H], f16, kind="ExternalInput")
    yT = nc.dram_tensor("yT", [128, HT, L], f32, kind="ExternalOutput")

    with tile.TileContext(nc) as tc:
        with (
            tc.tile_pool(name="const", bufs=1) as const,
            tc.tile_pool(name="w1p", bufs=4) as w1p,
            tc.tile_pool(name="w2p", bufs=4) as w2p,
            tc.tile_pool(name="xp", bufs=2) as xp,
            tc.tile_pool(name="hp", bufs=2) as hp,
            tc.tile_pool(name="op", bufs=2) as op,
            tc.tile_pool(name="psb", bufs=1, space="PSUM") as psb,
            tc.tile_pool(name="ps1", bufs=4, space="PSUM") as ps1,
            tc.tile_pool(name="ps2", bufs=2, space="PSUM") as ps2,
        ):
            # b1 arrives as [E_PER*FT, 128] rows; one PE transpose puts each
            # 128-wide bias row onto partitions: b1t[:, e*FT+f] is the [128,1]
            # per-partition bias column for F-tile f of expert e.
            ident = const.tile([E_PER * FT, E_PER * FT], f32)
            make_identity(nc, ident)
            b1rows = const.tile([E_PER * FT, 128], f32)
            nc.sync.dma_start(b1rows[:], b1.rearrange("e f p -> (e f) p"))
            b1ps = psb.tile([128, E_PER * FT], f32)
            nc.tensor.transpose(b1ps[:], b1rows[:], ident[:])
            b1t = const.tile([128, E_PER * FT], f32)
            nc.vector.tensor_copy(b1t[:], b1ps[:])

            def expert_pass():
                # one consolidated token load / output store per pass
                xall = xp.tile([128, HT, L], f16, tag="xall")
                nc.sync.dma_start(xall[:], xg[:])
                oall = op.tile([128, HT, L], f32, tag="oall")
                for e in range(E_PER):
                    c0, ce = off[e], cs[e]
                    w1t = w1p.tile([128, HT, F], f16, tag="w1t")
                    nc.sync.dma_start(w1t[:], w1[e])
                    w2t = w2p.tile([128, FT, H], f16, tag="w2t")
                    nc.sync.dma_start(w2t[:], w2[e])

                    # layer 1: hT[f*128:(f+1)*128, :] = gelu(w1.T @ xT + b1)
                    ht = hp.tile([128, FT, cmax], f16, tag="ht")
                    for f in range(FT):
                        ps = ps1.tile([128, cmax], f32, tag="ps1t")
                        for k in range(HT):
                            nc.tensor.matmul(
                                ps[:, :ce],
                                w1t[:, k, f * 128 : (f + 1) * 128],
                                xall[:, k, c0 : c0 + ce],
                                start=(k == 0),
                                stop=(k == HT - 1),
                            )
                        nc.scalar.activation(
                            ht[:, f, :ce],
                            ps[:, :ce],
                            AF.Gelu,
                            bias=b1t[:, e * FT + f : e * FT + f + 1],
                        )

                    # layer 2: yT[h*128:(h+1)*128, :] = w2.T @ hT  (b2 on host)
                    for hh in range(HT):
                        pso = ps2.tile([128, cmax], f32, tag="ps2t")
                        for k in range(FT):
                            nc.tensor.matmul(
                                pso[:, :ce],
                                w2t[:, k, hh * 128 : (hh + 1) * 128],
                                ht[:, k, :ce],
                                start=(k == 0),
                                stop=(k == FT - 1),
                            )
                        nc.vector.tensor_copy(oall[:, hh, c0 : c0 + ce], pso[:, :ce])
                nc.sync.dma_start(yT[:], oall[:])

            loop_ctx = tc.For_i(0, repeat, 1) if dyn_loop else nullcontext()
            with loop_ctx:
                for _ in range(1 if dyn_loop else repeat):
                    expert_pass()

    nc.compile()
    return nc


def _get_program(cs, repeat=1, dyn_loop=False):
    key = (tuple(cs), repeat, dyn_loop)
    if key not in _compiled:
        _compiled[key] = _build(list(cs), repeat, dyn_loop)
    return _compiled[key]


def _route(xf, w_router, b_router):
    """Replicate reference routing in fp32 numpy. Returns top-2 idx/gates."""
    logits = xf @ w_router + b_router  # [T, E] fp32
    m = logits.max(axis=-1, keepdims=True)
    p = np.exp(logits - m, dtype=np.float32)
    p /= p.sum(axis=-1, keepdims=True)
    rows = np.arange(T)
    i1 = p.argmax(axis=-1)
    p1 = p[rows, i1]
    pm = p.copy()
    pm[rows, i1] = -np.inf
    i2 = pm.argmax(axis=-1)
    p2 = p[rows, i2]
    denom = p1 + p2
    g1 = p1 / denom
    g2 = p2 / denom
    return i1, i2, g1.astype(np.float32), g2.astype(np.float32)


def run_device(in_maps, cs, repeat=1, dyn_loop=False):
    from concourse.bass_utils import run_bass_kernel_spmd

    nc = _get_program(cs, repeat, dyn_loop)
    res = run_bass_kernel_spmd(nc, in_maps, list(range(N_CORES)))
    return res


def prepare(x, w_router, b_router, w1, b1, w2, b2):
    """Host routing + dispatch. Returns (in_maps, combine_state)."""
    x = np.ascontiguousarray(np.asarray(x, dtype=np.float32))
    w_router = np.asarray(w_router, dtype=np.float32)
    b_router = np.asarray(b_router, dtype=np.float32)
    w1 = np.ascontiguousarray(np.asarray(w1, dtype=np.float32))
    b1 = np.ascontiguousarray(np.asarray(b1, dtype=np.float32))
    w2 = np.ascontiguousarray(np.asarray(w2, dtype=np.float32))
    b2 = np.asarray(b2, dtype=np.float32)

    xf = x.reshape(T, H)
    i1, i2, g1, g2 = _route(xf, w_router, b_router)

    ee = np.concatenate([i1, i2])  # expert of each (token, slot)
    tok = np.concatenate([np.arange(T, dtype=np.int32)] * 2)
    gg = np.concatenate([g1, g2])
    order = np.argsort(ee, kind="stable")
    tok_s, gg_s = tok[order], gg[order]
    counts = np.bincount(ee, minlength=E)
    starts = np.zeros(E + 1, dtype=np.int64)
    np.cumsum(counts, out=starts[1:])

    # snake-deal experts to (core, slot) by descending count so every core's
    # slot s has a similar load; slot capacity = max count in the slot.
    rank = np.argsort(-counts, kind="stable")  # expert ids, busiest first
    perm = np.empty((E_PER, N_CORES), dtype=np.int64)  # [slot, core] -> expert
    for s in range(E_PER):
        row = rank[s * N_CORES : (s + 1) * N_CORES]
        perm[s] = row if s % 2 == 0 else row[::-1]
    cs = [
        max(8, int(-(-int(counts[perm[s]].max()) // 8) * 8)) for s in range(E_PER)
    ]
    assert max(cs) <= 512, f"expert overflow: {max(cs)} > 512 unsupported"
    off = np.zeros(E_PER + 1, dtype=np.int64)
    np.cumsum(cs, out=off[1:])
    L = int(off[-1])

    w1r = w1.reshape(E, HT, 128, F).transpose(0, 2, 1, 3)  # [E,128,HT,F]
    w2r = w2.reshape(E, FT, 128, H).transpose(0, 2, 1, 3)  # [E,128,FT,H]
    b1r = b1.reshape(E, FT, 128)

    xfT16 = np.ascontiguousarray(xf.T.astype(np.float16))  # [H, T]

    in_maps = []
    for c in range(N_CORES):
        exps = perm[:, c]
        xcore = np.zeros((H, L), dtype=np.float16)
        for s, e in enumerate(exps):
            lo, hi = starts[e], starts[e + 1]
            if hi > lo:
                xcore[:, off[s] : off[s] + (hi - lo)] = xfT16[:, tok_s[lo:hi]]
        in_maps.append(
            {
                "xg": np.ascontiguousarray(
                    xcore.reshape(HT, 128, L).transpose(1, 0, 2)
                ),
                "w1": np.ascontiguousarray(w1r[exps].astype(np.float16)),
                "b1": np.ascontiguousarray(b1r[exps]),
                "w2": np.ascontiguousarray(w2r[exps].astype(np.float16)),
            }
        )
    state = dict(
        cs=cs,
        off=off,
        perm=perm,
        counts=counts,
        starts=starts,
        tok_s=tok_s,
        gg_s=gg_s,
        b2=b2,
    )
    return in_maps, state


def combine(results, state):
    """Scatter-add device outputs back to [B,S,H] + aux loss."""
    off, perm = state["off"], state["perm"]
    starts, tok_s, gg_s = state["starts"], state["tok_s"], state["gg_s"]
    b2 = state["b2"]
    L = int(off[-1])
    out = np.zeros((T, H), dtype=np.float32)
    for c in range(N_CORES):
        # yT [128, HT, L] -> [H, L]
        yTc = results[c]["yT"].transpose(1, 0, 2).reshape(H, L)
        for s in range(E_PER):
            e = int(perm[s, c])
            lo, hi = starts[e], starts[e + 1]
            if hi > lo:
                y = yTc[:, off[s] : off[s] + (hi - lo)].T  # [cnt, H]
                g = gg_s[lo:hi].astype(np.float32)[:, None]
                out[tok_s[lo:hi]] += g * (y + b2[e][None, :])
    output = out.reshape(B, S, H)

    usage = (state["counts"] / np.float32(2 * T)).astype(np.float32)
    target = np.full((E,), np.float32(1.0) / np.float32(E), dtype=np.float32)
    lb_loss = np.float32(np.mean((usage - target) ** 2, dtype=np.float32) * 0.01)
    return output, lb_loss


def kernel(x, w_router, b_router, w1, b1, w2, b2):
    in_maps, state = prepare(x, w_router, b_router, w1, b1, w2, b2)
    res = run_device(in_maps, state["cs"], repeat=1)
    return combine(res.results, state)
